# revision 16
# baseline (speedup 1.0000x reference)
"""Dilated LSTM (B=8, T=2048, C=1024, H=1024, D=4) on 8 trn2 NeuronCores.

Strategy: data-parallel over batch (core c <- batch item c, its 4 dilation
chains). Everything is core-local (no cross-core traffic; remote-DMA and
collectives are not usable in this deployment).

Per core:
  Phase A (GEMM): xg[t, g] = x[t, :] @ W_ih^T + (b_ih + b_hh), computed as
    9 K-tiles of 128 (the 9th K-tile is a host-appended ones-row carrying the
    bias), staged to DRAM (32 MB).
  Phase B (recurrence): 512 steps. Per step l, gates for the 4 chains:
    G[4, 4096] = hT_pack-stationary x W_hh^T-moving matmuls (8 K-tiles x 8
    PSUM chunks of 512), then DVE adds xg, ACT sigmoid/tanh, DVE/ACT c/h
    update, PE transposes h[4,1024] -> hT_pack[128, 8x4] for the next step.

Host reassembles y[b, t, h] from per-core y slabs.
"""

import sys

sys.path.insert(0, "/opt/trn_rl_repo")

import numpy as np

B, T, C, H, D = 8, 2048, 1024, 1024, 4
L = T // D  # 512 steps
G4 = 4 * H  # 4096 gates
KT = C // 128  # 8 k-tiles for x / h
NCORES = 8

_cached = {}

import os

N_STEPS = int(os.environ.get("DLSTM_STEPS", str(L)))  # dev override only
NO_DMA = os.environ.get("DLSTM_NODMA", "0") == "1"  # timing experiment only


def _build(w_dtype_str):
    import concourse.bass as bass
    import concourse.bacc as bacc
    import concourse.mybir as mybir

    F32 = mybir.dt.float32
    WDT = F32 if w_dtype_str == "f32" else mybir.dt.bfloat16

    nc = bacc.Bacc(None, target_bir_lowering=False)

    # ---- I/O ----
    xT = nc.dram_tensor("xT", [C + 128, T], F32, kind="ExternalInput")
    wihT = nc.dram_tensor("wihT", [C + 128, G4], F32, kind="ExternalInput")
    whhT = nc.dram_tensor("whhT", [C, G4], WDT, kind="ExternalInput")
    ident = nc.dram_tensor("ident", [4, 4], F32, kind="ExternalInput")
    y = nc.dram_tensor("y", [T, H], F32, kind="ExternalOutput")
    xg = nc.dram_tensor("xg", [T, G4], F32)  # internal staging, 32MB

    KTX = KT + 1  # 9 k-tiles incl bias row block

    from contextlib import ExitStack

    with ExitStack() as es_top:
        sems = {}
        for nm in ("ld_sem gp_sem ev_sem xw0_sem xw1_sem wh_sem ms_sem ps_sem ds_sem "
                   "as_sem cs_sem hs_sem tp_sem es_sem xr_sem yw0_sem yw1_sem "
                   "c2s_sem igs_sem").split():
            sems[nm] = es_top.enter_context(nc.semaphore(nm))
        ld_sem, gp_sem, ev_sem, wh_sem, ms_sem, ps_sem, ds_sem = (
            sems["ld_sem"], sems["gp_sem"], sems["ev_sem"],
            sems["wh_sem"], sems["ms_sem"], sems["ps_sem"], sems["ds_sem"])
        as_sem, cs_sem, hs_sem, tp_sem, es_sem, xr_sem = (
            sems["as_sem"], sems["cs_sem"], sems["hs_sem"], sems["tp_sem"],
            sems["es_sem"], sems["xr_sem"])
        xw_sems = (sems["xw0_sem"], sems["xw1_sem"])
        c2s_sem, igs_sem = sems["c2s_sem"], sems["igs_sem"]
        yw_sems = (sems["yw0_sem"], sems["yw1_sem"])
        # ---------------- Phase A: input GEMM ----------------
        with ExitStack() as es_a:
            xT_sb = es_a.enter_context(nc.sbuf_tensor("xT_sb", [128, KTX * T], F32))
            wih_sb = es_a.enter_context(nc.sbuf_tensor("wih_sb", [128, KTX * 2048], F32))
            stage = es_a.enter_context(nc.sbuf_tensor("stage", [128, 2 * 512], F32))
            gps = es_a.enter_context(nc.psum_tensor("gps", [128, 2 * 512], F32))
            blk = es_a.enter_context(nc.Block())
            # xT_sb k-tile k at cols [k*T, (k+1)*T); wih_sb k-tile at [k*2048, ..)
            @blk.sync
            def _(s):
                for k in range(KTX):
                    s.dma_start(
                        xT_sb[:, k * T : (k + 1) * T], xT[k * 128 : (k + 1) * 128, :]
                    ).then_inc(ld_sem, 16)
                # pass 0 wih halves
                for k in range(KTX):
                    s.dma_start(
                        wih_sb[:, k * 2048 : (k + 1) * 2048],
                        wihT[k * 128 : (k + 1) * 128, 0:2048],
                    ).then_inc(ld_sem, 16)
                # pass 1 loads gated on pass-0 compute done
                s.wait_ge(gp_sem, 64)
                for k in range(KTX):
                    s.dma_start(
                        wih_sb[:, k * 2048 : (k + 1) * 2048],
                        wihT[k * 128 : (k + 1) * 128, 2048:4096],
                    ).then_inc(ld_sem, 16)

            @blk.tensor
            def _(t):
                for p in range(2):
                    t.wait_ge(ld_sem, 16 * KTX * (2 + p))
                    for m in range(16):  # bt tiles of 128
                        for ns in range(4):  # 512-col chunks within the half
                            idx = p * 64 + m * 4 + ns
                            if idx >= 2:
                                t.wait_ge(ev_sem, idx - 1)
                            bank = idx % 2
                            for k in range(KTX):
                                t.matmul(
                                    gps[:, bank * 512 : (bank + 1) * 512],
                                    xT_sb[:, k * T + m * 128 : k * T + (m + 1) * 128],
                                    wih_sb[:, k * 2048 + ns * 512 : k * 2048 + (ns + 1) * 512],
                                    start=(k == 0),
                                    stop=(k == KTX - 1),
                                ).then_inc(gp_sem, 1) if k == KTX - 1 else t.matmul(
                                    gps[:, bank * 512 : (bank + 1) * 512],
                                    xT_sb[:, k * T + m * 128 : k * T + (m + 1) * 128],
                                    wih_sb[:, k * 2048 + ns * 512 : k * 2048 + (ns + 1) * 512],
                                    start=(k == 0),
                                    stop=(k == KTX - 1),
                                )

            @blk.vector
            def _(v):
                for p in range(2):
                    for m in range(16):
                        for ns in range(4):
                            idx = p * 64 + m * 4 + ns
                            bank = idx % 2
                            v.wait_ge(gp_sem, idx + 1)
                            if idx >= 2:
                                v.wait_ge(xw_sems[idx % 2], 16 * (idx // 2))
                            v.tensor_copy(
                                stage[:, bank * 512 : (bank + 1) * 512],
                                gps[:, bank * 512 : (bank + 1) * 512],
                            ).then_inc(ev_sem, 1)

            @blk.scalar
            def _(s):
                for p in range(2):
                    for m in range(16):
                        for ns in range(4):
                            idx = p * 64 + m * 4 + ns
                            bank = idx % 2
                            s.wait_ge(ev_sem, idx + 1)
                            s.dma_start(
                                xg[m * 128 : (m + 1) * 128, p * 2048 + ns * 512 : p * 2048 + (ns + 1) * 512],
                                stage[:, bank * 512 : (bank + 1) * 512],
                            ).then_inc(xw_sems[idx % 2], 16)
                s.wait_ge(xw_sems[0], 16 * 64)
                s.wait_ge(xw_sems[1], 16 * 64)

        # ---------------- Phase B: recurrence ----------------
        # wait: the matmul group semaphore convention above double-emits; see
        # the tensor block - it emits exactly one matmul per (k), with
        # then_inc only on the last. (Python ternary keeps one instruction.)
        LS = N_STEPS
        with ExitStack() as es_b:
            whh_sb = es_b.enter_context(nc.sbuf_tensor("whh_sb", [128, KT * G4], WDT))
            hT_pack = es_b.enter_context(nc.sbuf_tensor("hT_pack", [128, KT * 32], WDT))
            id_sb = es_b.enter_context(nc.sbuf_tensor("id_sb", [4, 4], F32))
            g_sb = es_b.enter_context(nc.sbuf_tensor("g_sb", [4, G4], F32))
            xg_sb = es_b.enter_context(nc.sbuf_tensor("xg_sb", [4, G4], F32))
            c_sb = es_b.enter_context(nc.sbuf_tensor("c_sb", [4, H], F32))
            c2_sb = es_b.enter_context(nc.sbuf_tensor("c2_sb", [4, H], F32))
            t_sb = es_b.enter_context(nc.sbuf_tensor("t_sb", [4, H], F32))
            h_sb = es_b.enter_context(nc.sbuf_tensor("h_sb", [4, 2 * H], F32))
            G_ps = es_b.enter_context(nc.psum_tensor("G_ps", [4, 6 * 512], F32))
            TA_ps = es_b.enter_context(nc.psum_tensor("TA_ps", [128, 4], F32))
            TB_ps = es_b.enter_context(nc.psum_tensor("TB_ps", [128, 4], F32))
            blk = es_b.enter_context(nc.Block())
            SIG = mybir.ActivationFunctionType.Sigmoid
            TANH = mybir.ActivationFunctionType.Tanh

            def gbank(n):  # psum column base for gate chunk n (chunks 6,7 alias 0,1)
                return (n % 6) * 512

            @blk.sync
            def _(s):
                for k in range(KT):
                    s.dma_start(
                        whh_sb[:, k * G4 : (k + 1) * G4], whhT[k * 128 : (k + 1) * 128, :]
                    ).then_inc(wh_sem, 16)
                s.dma_start(id_sb[:, :], ident[:, :]).then_inc(wh_sem, 16)
                for l in range(LS if not NO_DMA else 1):
                    if l >= 1:
                        # single xg buffer: refill after step l-1's adds consumed it
                        s.wait_ge(ds_sem, 8 * l)
                    s.dma_start(xg_sb[:, :], xg[4 * l : 4 * l + 4, :]).then_inc(
                        xr_sem, 16
                    )

            @blk.scalar
            def _(s):
                # y writeback on the scalar (HWDGE) queue + the ACT work
                for l in range(LS):
                    # ACT: A1 sigmoid(i,f), A2 tanh(g), A3 sigmoid(o), A4 tanh(c)
                    s.wait_ge(ds_sem, 8 * l + 4)
                    s.activation(g_sb[:, 0:2048], g_sb[:, 0:2048], SIG).then_inc(as_sem, 1)
                    s.wait_ge(ds_sem, 8 * l + 6)
                    s.activation(g_sb[:, 2048:3072], g_sb[:, 2048:3072], TANH).then_inc(as_sem, 1)
                    s.wait_ge(ds_sem, 8 * l + 8)
                    s.activation(g_sb[:, 3072:4096], g_sb[:, 3072:4096], SIG).then_inc(as_sem, 1)
                    s.wait_ge(cs_sem, l + 1)
                    s.activation(t_sb[:, :], c_sb[:, :], TANH).then_inc(as_sem, 1)
                    # y writeback of h(l)
                    if NO_DMA and l != LS - 1:
                        continue
                    s.wait_ge(hs_sem, l + 1)
                    s.dma_start(
                        y[4 * l : 4 * l + 4, :], h_sb[:, (l % 2) * H : (l % 2 + 1) * H]
                    ).then_inc(yw_sems[l % 2], 16)
                if not NO_DMA:
                    s.wait_ge(yw_sems[0], 16 * ((LS + 1) // 2))
                    s.wait_ge(yw_sems[1], 16 * (LS // 2))

            @blk.gpsimd
            def _(g):
                g.memset(hT_pack[:, :], 0.0)
                g.memset(c_sb[:, :], 0.0).then_inc(ms_sem, 1)

            @blk.tensor
            def _(t):
                t.wait_ge(wh_sem, 16 * (KT + 1))
                t.wait_ge(ms_sem, 1)
                for l in range(LS):
                    if l >= 1:
                        t.wait_ge(es_sem, 8 * l)  # hT(l-1) fully evacuated
                    if l >= 1:
                        t.wait_ge(ds_sem, 8 * (l - 1) + 6)  # banks 0..5 free
                    for n in range(8):
                        if n >= 6:
                            t.wait_ge(ds_sem, 8 * l + (n - 6) + 1)
                        for k in range(KT):
                            mm = t.matmul(
                                G_ps[:, gbank(n) : gbank(n) + 512],
                                hT_pack[:, k * 32 : k * 32 + 4],
                                whh_sb[:, k * G4 + n * 512 : k * G4 + (n + 1) * 512],
                                start=(k == 0),
                                stop=(k == KT - 1),
                            )
                            if k == KT - 1:
                                mm.then_inc(ps_sem, 1)
                    # transposes of h(l) for next step
                    t.wait_ge(hs_sem, l + 1)
                    for kk in range(KT):
                        if kk >= 2:
                            t.wait_ge(es_sem, 8 * l + kk - 1)
                        tp = TA_ps if kk % 2 == 0 else TB_ps
                        t.transpose(
                            tp[:, 0:4],
                            h_sb[:, (l % 2) * H + kk * 128 : (l % 2) * H + (kk + 1) * 128],
                            id_sb[:, :],
                        ).then_inc(tp_sem, 1)

            @blk.vector
            def _(v):
                for l in range(LS):
                    # gate adds: g = G_ps + xg
                    for n in range(8):
                        v.wait_ge(ps_sem, 8 * l + n + 1)
                        if n == 0:
                            v.wait_ge(xr_sem, 16 * ((l + 1) if not NO_DMA else 1))
                        v.tensor_add(
                            g_sb[:, n * 512 : (n + 1) * 512],
                            G_ps[:, gbank(n) : gbank(n) + 512],
                            xg_sb[:, n * 512 : (n + 1) * 512],
                        ).then_inc(ds_sem, 1)
                    # c2 = f * c
                    v.wait_ge(as_sem, 4 * l + 1)
                    if l >= 1:
                        v.wait_ge(cs_sem, l)  # c_sb from step l-1 retired
                    v.tensor_mul(c2_sb[:, :], g_sb[:, 1024:2048], c_sb[:, :]).then_inc(
                        c2s_sem, 1
                    )
                    # t_sb reused as ig scratch: ig = i * g
                    v.wait_ge(as_sem, 4 * l + 2)
                    v.tensor_mul(t_sb[:, :], g_sb[:, 0:1024], g_sb[:, 2048:3072]).then_inc(
                        igs_sem, 1
                    )
                    # c = c2 + ig (same-engine RAW needs explicit sems)
                    v.wait_ge(c2s_sem, l + 1)
                    v.wait_ge(igs_sem, l + 1)
                    v.tensor_add(c_sb[:, :], c2_sb[:, :], t_sb[:, :]).then_inc(cs_sem, 1)
                    # h = o * tanh(c)
                    v.wait_ge(as_sem, 4 * l + 4)
                    if l >= 2 and not NO_DMA:
                        v.wait_ge(yw_sems[l % 2], 16 * (l // 2))
                    v.tensor_mul(
                        h_sb[:, (l % 2) * H : (l % 2 + 1) * H], g_sb[:, 3072:4096], t_sb[:, :]
                    ).then_inc(hs_sem, 1)
                    # hT evacs
                    for kk in range(KT):
                        v.wait_ge(tp_sem, 8 * l + kk + 1)
                        tp = TA_ps if kk % 2 == 0 else TB_ps
                        v.tensor_copy(hT_pack[:, kk * 32 : kk * 32 + 4], tp[:, 0:4]).then_inc(
                            es_sem, 1
                        )

    nc.finalize()
    return nc


def _get_nc(w_dtype_str):
    if w_dtype_str not in _cached:
        _cached[w_dtype_str] = _build(w_dtype_str)
    return _cached[w_dtype_str]


W_DTYPE = os.environ.get("DLSTM_WDT", "f32")


def kernel(x, W_ih, W_hh, b_ih, b_hh):
    from concourse.bass_utils import run_bass_kernel_spmd
    import ml_dtypes

    x = np.asarray(x, np.float32)
    W_ih = np.asarray(W_ih, np.float32)
    W_hh = np.asarray(W_hh, np.float32)
    bias = (np.asarray(b_ih, np.float32) + np.asarray(b_hh, np.float32))

    nc = _get_nc(W_DTYPE)

    # host-side prep
    wihT_ext = np.zeros((C + 128, G4), np.float32)
    wihT_ext[:C] = W_ih.T
    wihT_ext[C] = bias
    whhT = W_hh.T.copy()
    if W_DTYPE == "bf16":
        whhT = whhT.astype(ml_dtypes.bfloat16)
    ident = np.eye(4, dtype=np.float32)

    in_maps = []
    for c in range(NCORES):
        xT_ext = np.zeros((C + 128, T), np.float32)
        xT_ext[:C] = x[c].T
        xT_ext[C] = 1.0
        in_maps.append(
            {"xT": xT_ext, "wihT": wihT_ext, "whhT": whhT, "ident": ident}
        )

    res = run_bass_kernel_spmd(nc, in_maps, list(range(NCORES)))
    out = np.stack([res.results[c]["y"] for c in range(NCORES)], axis=0)
    return out.astype(np.float32)


# revision 18
# speedup vs baseline: 548.1947x; 548.1947x over previous
"""Dilated LSTM (B=8, T=2048, C=1024, H=1024, D=4) on 8 trn2 NeuronCores.

Strategy: data-parallel over batch (core c <- batch item c, its 4 dilation
chains). Everything is core-local (no cross-core traffic; remote-DMA and
collectives are not usable in this deployment).

Per core:
  Phase A (GEMM): xg[t, g] = x[t, :] @ W_ih^T + (b_ih + b_hh), computed as
    9 K-tiles of 128 (the 9th K-tile is a host-appended ones-row carrying the
    bias), staged to DRAM (32 MB).
  Phase B (recurrence): 512 steps. Per step l, gates for the 4 chains:
    G[4, 4096] = hT_pack-stationary x W_hh^T-moving matmuls (8 K-tiles x 8
    PSUM chunks of 512), then DVE adds xg, ACT sigmoid/tanh, DVE/ACT c/h
    update, PE transposes h[4,1024] -> hT_pack[128, 8x4] for the next step.

Host reassembles y[b, t, h] from per-core y slabs.
"""

import sys

sys.path.insert(0, "/opt/trn_rl_repo")

import numpy as np

B, T, C, H, D = 8, 2048, 1024, 1024, 4
L = T // D  # 512 steps
G4 = 4 * H  # 4096 gates
KT = C // 128  # 8 k-tiles for x / h
NCORES = 8

_cached = {}

import os

N_STEPS = int(os.environ.get("DLSTM_STEPS", str(L)))  # dev override only
NO_DMA = os.environ.get("DLSTM_NODMA", "0") == "1"  # timing experiment only


def _build(w_dtype_str):
    import concourse.bass as bass
    import concourse.bacc as bacc
    import concourse.mybir as mybir

    F32 = mybir.dt.float32
    WDT = F32 if w_dtype_str == "f32" else mybir.dt.bfloat16

    nc = bacc.Bacc(None, target_bir_lowering=False)

    # ---- I/O ----
    xT = nc.dram_tensor("xT", [C + 128, T], F32, kind="ExternalInput")
    wihT = nc.dram_tensor("wihT", [C + 128, G4], F32, kind="ExternalInput")
    whhT = nc.dram_tensor("whhT", [C, G4], WDT, kind="ExternalInput")
    ident = nc.dram_tensor("ident", [4, 4], F32, kind="ExternalInput")
    y = nc.dram_tensor("y", [T, H], F32, kind="ExternalOutput")
    xg = nc.dram_tensor("xg", [T, G4], F32)  # internal staging, 32MB

    KTX = KT + 1  # 9 k-tiles incl bias row block

    from contextlib import ExitStack

    with ExitStack() as es_top:
        sems = {}
        for nm in ("ld_sem gp_sem ev_sem xw0_sem xw1_sem wh_sem ms_sem ps_sem ds_sem "
                   "as_sem cs_sem hs_sem tp_sem es_sem xr_sem yw0_sem yw1_sem "
                   "c2s_sem igs_sem").split():
            sems[nm] = es_top.enter_context(nc.semaphore(nm))
        ld_sem, gp_sem, ev_sem, wh_sem, ms_sem, ps_sem, ds_sem = (
            sems["ld_sem"], sems["gp_sem"], sems["ev_sem"],
            sems["wh_sem"], sems["ms_sem"], sems["ps_sem"], sems["ds_sem"])
        as_sem, cs_sem, hs_sem, tp_sem, es_sem, xr_sem = (
            sems["as_sem"], sems["cs_sem"], sems["hs_sem"], sems["tp_sem"],
            sems["es_sem"], sems["xr_sem"])
        xw_sems = (sems["xw0_sem"], sems["xw1_sem"])
        c2s_sem, igs_sem = sems["c2s_sem"], sems["igs_sem"]
        yw_sems = (sems["yw0_sem"], sems["yw1_sem"])
        # ---------------- Phase A: input GEMM ----------------
        with ExitStack() as es_a:
            xT_sb = es_a.enter_context(nc.sbuf_tensor("xT_sb", [128, KTX * T], F32))
            wih_sb = es_a.enter_context(nc.sbuf_tensor("wih_sb", [128, KTX * 2048], F32))
            stage = es_a.enter_context(nc.sbuf_tensor("stage", [128, 2 * 512], F32))
            gps = es_a.enter_context(nc.psum_tensor("gps", [128, 2 * 512], F32))
            blk = es_a.enter_context(nc.Block())
            # xT_sb k-tile k at cols [k*T, (k+1)*T); wih_sb k-tile at [k*2048, ..)
            @blk.sync
            def _(s):
                for k in range(KTX):
                    s.dma_start(
                        xT_sb[:, k * T : (k + 1) * T], xT[k * 128 : (k + 1) * 128, :]
                    ).then_inc(ld_sem, 16)
                # pass 0 wih halves
                for k in range(KTX):
                    s.dma_start(
                        wih_sb[:, k * 2048 : (k + 1) * 2048],
                        wihT[k * 128 : (k + 1) * 128, 0:2048],
                    ).then_inc(ld_sem, 16)
                # pass 1 loads gated on pass-0 compute done
                s.wait_ge(gp_sem, 64)
                for k in range(KTX):
                    s.dma_start(
                        wih_sb[:, k * 2048 : (k + 1) * 2048],
                        wihT[k * 128 : (k + 1) * 128, 2048:4096],
                    ).then_inc(ld_sem, 16)

            @blk.tensor
            def _(t):
                for p in range(2):
                    t.wait_ge(ld_sem, 16 * KTX * (2 + p))
                    for m in range(16):  # bt tiles of 128
                        for ns in range(4):  # 512-col chunks within the half
                            idx = p * 64 + m * 4 + ns
                            if idx >= 2:
                                t.wait_ge(ev_sem, idx - 1)
                            bank = idx % 2
                            for k in range(KTX):
                                t.matmul(
                                    gps[:, bank * 512 : (bank + 1) * 512],
                                    xT_sb[:, k * T + m * 128 : k * T + (m + 1) * 128],
                                    wih_sb[:, k * 2048 + ns * 512 : k * 2048 + (ns + 1) * 512],
                                    start=(k == 0),
                                    stop=(k == KTX - 1),
                                ).then_inc(gp_sem, 1) if k == KTX - 1 else t.matmul(
                                    gps[:, bank * 512 : (bank + 1) * 512],
                                    xT_sb[:, k * T + m * 128 : k * T + (m + 1) * 128],
                                    wih_sb[:, k * 2048 + ns * 512 : k * 2048 + (ns + 1) * 512],
                                    start=(k == 0),
                                    stop=(k == KTX - 1),
                                )

            @blk.vector
            def _(v):
                for p in range(2):
                    for m in range(16):
                        for ns in range(4):
                            idx = p * 64 + m * 4 + ns
                            bank = idx % 2
                            v.wait_ge(gp_sem, idx + 1)
                            if idx >= 2:
                                v.wait_ge(xw_sems[idx % 2], 16 * (idx // 2))
                            v.tensor_copy(
                                stage[:, bank * 512 : (bank + 1) * 512],
                                gps[:, bank * 512 : (bank + 1) * 512],
                            ).then_inc(ev_sem, 1)

            @blk.scalar
            def _(s):
                for p in range(2):
                    for m in range(16):
                        for ns in range(4):
                            idx = p * 64 + m * 4 + ns
                            bank = idx % 2
                            s.wait_ge(ev_sem, idx + 1)
                            s.dma_start(
                                xg[m * 128 : (m + 1) * 128, p * 2048 + ns * 512 : p * 2048 + (ns + 1) * 512],
                                stage[:, bank * 512 : (bank + 1) * 512],
                            ).then_inc(xw_sems[idx % 2], 16)
                s.wait_ge(xw_sems[0], 16 * 64)
                s.wait_ge(xw_sems[1], 16 * 64)

        # ---------------- Phase B: recurrence ----------------
        # wait: the matmul group semaphore convention above double-emits; see
        # the tensor block - it emits exactly one matmul per (k), with
        # then_inc only on the last. (Python ternary keeps one instruction.)
        LS = N_STEPS
        with ExitStack() as es_b:
            whh_sb = es_b.enter_context(nc.sbuf_tensor("whh_sb", [128, KT * G4], WDT))
            hT_pack = es_b.enter_context(nc.sbuf_tensor("hT_pack", [128, KT * 32], WDT))
            id_sb = es_b.enter_context(nc.sbuf_tensor("id_sb", [4, 4], F32))
            g_sb = es_b.enter_context(nc.sbuf_tensor("g_sb", [4, G4], F32))
            xg_sb = es_b.enter_context(nc.sbuf_tensor("xg_sb", [4, G4], F32))
            c_sb = es_b.enter_context(nc.sbuf_tensor("c_sb", [4, H], F32))
            c2_sb = es_b.enter_context(nc.sbuf_tensor("c2_sb", [4, H], F32))
            t_sb = es_b.enter_context(nc.sbuf_tensor("t_sb", [4, H], F32))
            h_sb = es_b.enter_context(nc.sbuf_tensor("h_sb", [4, 2 * H], F32))
            G_ps = es_b.enter_context(nc.psum_tensor("G_ps", [4, 6 * 512], F32))
            TA_ps = es_b.enter_context(nc.psum_tensor("TA_ps", [128, 4], F32))
            TB_ps = es_b.enter_context(nc.psum_tensor("TB_ps", [128, 4], F32))
            blk = es_b.enter_context(nc.Block())
            SIG = mybir.ActivationFunctionType.Sigmoid
            TANH = mybir.ActivationFunctionType.Tanh

            def gbank(n):  # psum column base for gate chunk n (chunks 6,7 alias 0,1)
                return (n % 6) * 512

            @blk.sync
            def _(s):
                for k in range(KT):
                    s.dma_start(
                        whh_sb[:, k * G4 : (k + 1) * G4], whhT[k * 128 : (k + 1) * 128, :]
                    ).then_inc(wh_sem, 16)
                s.dma_start(id_sb[:, :], ident[:, :]).then_inc(wh_sem, 16)
                for l in range(LS if not NO_DMA else 1):
                    if l >= 1:
                        # single xg buffer: refill after step l-1's adds consumed it
                        s.wait_ge(ds_sem, 8 * l)
                    s.dma_start(xg_sb[:, :], xg[4 * l : 4 * l + 4, :]).then_inc(
                        xr_sem, 16
                    )

            @blk.scalar
            def _(s):
                # y writeback on the scalar (HWDGE) queue + the ACT work
                for l in range(LS):
                    # ACT: A1 sigmoid(i,f), A2 tanh(g), A3 sigmoid(o), A4 tanh(c)
                    s.wait_ge(ds_sem, 8 * l + 4)
                    s.activation(g_sb[:, 0:2048], g_sb[:, 0:2048], SIG).then_inc(as_sem, 1)
                    s.wait_ge(ds_sem, 8 * l + 6)
                    s.activation(g_sb[:, 2048:3072], g_sb[:, 2048:3072], TANH).then_inc(as_sem, 1)
                    s.wait_ge(ds_sem, 8 * l + 8)
                    s.activation(g_sb[:, 3072:4096], g_sb[:, 3072:4096], SIG).then_inc(as_sem, 1)
                    s.wait_ge(cs_sem, l + 1)
                    s.activation(t_sb[:, :], c_sb[:, :], TANH).then_inc(as_sem, 1)
                    # y writeback of h(l)
                    if NO_DMA and l != LS - 1:
                        continue
                    s.wait_ge(hs_sem, l + 1)
                    s.dma_start(
                        y[4 * l : 4 * l + 4, :], h_sb[:, (l % 2) * H : (l % 2 + 1) * H]
                    ).then_inc(yw_sems[l % 2], 16)
                if not NO_DMA:
                    s.wait_ge(yw_sems[0], 16 * ((LS + 1) // 2))
                    s.wait_ge(yw_sems[1], 16 * (LS // 2))

            @blk.gpsimd
            def _(g):
                g.memset(hT_pack[:, :], 0.0)
                g.memset(c_sb[:, :], 0.0).then_inc(ms_sem, 1)

            @blk.tensor
            def _(t):
                t.wait_ge(wh_sem, 16 * (KT + 1))
                t.wait_ge(ms_sem, 1)
                for l in range(LS):
                    if l >= 1:
                        t.wait_ge(es_sem, 8 * l)  # hT(l-1) fully evacuated
                    if l >= 1:
                        t.wait_ge(ds_sem, 8 * (l - 1) + 6)  # banks 0..5 free
                    for n in range(8):
                        if n >= 6:
                            t.wait_ge(ds_sem, 8 * l + (n - 6) + 1)
                        for k in range(KT):
                            mm = t.matmul(
                                G_ps[:, gbank(n) : gbank(n) + 512],
                                hT_pack[:, k * 32 : k * 32 + 4],
                                whh_sb[:, k * G4 + n * 512 : k * G4 + (n + 1) * 512],
                                start=(k == 0),
                                stop=(k == KT - 1),
                            )
                            if k == KT - 1:
                                mm.then_inc(ps_sem, 1)
                    # transposes of h(l) for next step
                    t.wait_ge(hs_sem, l + 1)
                    for kk in range(KT):
                        if kk >= 2:
                            t.wait_ge(es_sem, 8 * l + kk - 1)
                        tp = TA_ps if kk % 2 == 0 else TB_ps
                        t.transpose(
                            tp[:, 0:4],
                            h_sb[:, (l % 2) * H + kk * 128 : (l % 2) * H + (kk + 1) * 128],
                            id_sb[:, :],
                        ).then_inc(tp_sem, 1)

            @blk.vector
            def _(v):
                for l in range(LS):
                    # gate adds: g = G_ps + xg
                    for n in range(8):
                        v.wait_ge(ps_sem, 8 * l + n + 1)
                        if n == 0:
                            v.wait_ge(xr_sem, 16 * ((l + 1) if not NO_DMA else 1))
                        v.tensor_add(
                            g_sb[:, n * 512 : (n + 1) * 512],
                            G_ps[:, gbank(n) : gbank(n) + 512],
                            xg_sb[:, n * 512 : (n + 1) * 512],
                        ).then_inc(ds_sem, 1)
                    # c2 = f * c
                    v.wait_ge(as_sem, 4 * l + 1)
                    if l >= 1:
                        v.wait_ge(cs_sem, l)  # c_sb from step l-1 retired
                    v.tensor_mul(c2_sb[:, :], g_sb[:, 1024:2048], c_sb[:, :]).then_inc(
                        c2s_sem, 1
                    )
                    # t_sb reused as ig scratch: ig = i * g
                    v.wait_ge(as_sem, 4 * l + 2)
                    v.tensor_mul(t_sb[:, :], g_sb[:, 0:1024], g_sb[:, 2048:3072]).then_inc(
                        igs_sem, 1
                    )
                    # c = c2 + ig (same-engine RAW needs explicit sems)
                    v.wait_ge(c2s_sem, l + 1)
                    v.wait_ge(igs_sem, l + 1)
                    v.tensor_add(c_sb[:, :], c2_sb[:, :], t_sb[:, :]).then_inc(cs_sem, 1)
                    # h = o * tanh(c)
                    v.wait_ge(as_sem, 4 * l + 4)
                    if l >= 2 and not NO_DMA:
                        v.wait_ge(yw_sems[l % 2], 16 * (l // 2))
                    v.tensor_mul(
                        h_sb[:, (l % 2) * H : (l % 2 + 1) * H], g_sb[:, 3072:4096], t_sb[:, :]
                    ).then_inc(hs_sem, 1)
                    # hT evacs
                    for kk in range(KT):
                        v.wait_ge(tp_sem, 8 * l + kk + 1)
                        tp = TA_ps if kk % 2 == 0 else TB_ps
                        v.tensor_copy(hT_pack[:, kk * 32 : kk * 32 + 4], tp[:, 0:4]).then_inc(
                            es_sem, 1
                        )

    nc.finalize()
    return nc


def _get_nc(w_dtype_str):
    if w_dtype_str not in _cached:
        _cached[w_dtype_str] = _build(w_dtype_str)
    return _cached[w_dtype_str]


W_DTYPE = os.environ.get("DLSTM_WDT", "f32")


def kernel(x, W_ih, W_hh, b_ih, b_hh):
    from concourse.bass_utils import run_bass_kernel_spmd
    import ml_dtypes

    x = np.asarray(x, np.float32)
    W_ih = np.asarray(W_ih, np.float32)
    W_hh = np.asarray(W_hh, np.float32)
    bias = (np.asarray(b_ih, np.float32) + np.asarray(b_hh, np.float32))

    nc = _get_nc(W_DTYPE)

    # host-side prep
    wihT_ext = np.zeros((C + 128, G4), np.float32)
    wihT_ext[:C] = W_ih.T
    wihT_ext[C] = bias
    whhT = W_hh.T.copy()
    if W_DTYPE == "bf16":
        whhT = whhT.astype(ml_dtypes.bfloat16)
    ident = np.eye(4, dtype=np.float32)

    in_maps = []
    for c in range(NCORES):
        xT_ext = np.zeros((C + 128, T), np.float32)
        xT_ext[:C] = x[c].T
        xT_ext[C] = 1.0
        in_maps.append(
            {"xT": xT_ext, "wihT": wihT_ext, "whhT": whhT, "ident": ident}
        )

    res = run_bass_kernel_spmd(nc, in_maps, list(range(NCORES)))
    out = np.stack([res.results[c]["y"] for c in range(NCORES)], axis=0)
    return out.astype(np.float32)


# revision 19
# speedup vs baseline: 554.1280x; 1.0108x over previous
"""Dilated LSTM (B=8, T=2048, C=1024, H=1024, D=4) on 8 trn2 NeuronCores.

Strategy: data-parallel over batch (core c <- batch item c, its 4 dilation
chains). Everything is core-local (no cross-core traffic; remote-DMA and
collectives are not usable in this deployment).

Per core:
  Phase A (GEMM): xg[t, g] = x[t, :] @ W_ih^T + (b_ih + b_hh), computed as
    9 K-tiles of 128 (the 9th K-tile is a host-appended ones-row carrying the
    bias), staged to DRAM (32 MB).
  Phase B (recurrence): 512 steps. Per step l, gates for the 4 chains:
    G[4, 4096] = hT_pack-stationary x W_hh^T-moving matmuls (8 K-tiles x 8
    PSUM chunks of 512), then DVE adds xg, ACT sigmoid/tanh, DVE/ACT c/h
    update, PE transposes h[4,1024] -> hT_pack[128, 8x4] for the next step.

Host reassembles y[b, t, h] from per-core y slabs.
"""

import sys

sys.path.insert(0, "/opt/trn_rl_repo")

import numpy as np

B, T, C, H, D = 8, 2048, 1024, 1024, 4
L = T // D  # 512 steps
G4 = 4 * H  # 4096 gates
KT = C // 128  # 8 k-tiles for x / h
NCORES = 8

_cached = {}

import os

N_STEPS = int(os.environ.get("DLSTM_STEPS", str(L)))  # dev override only
NO_DMA = os.environ.get("DLSTM_NODMA", "0") == "1"  # timing experiment only


def _build(w_dtype_str):
    import concourse.bass as bass
    import concourse.bacc as bacc
    import concourse.mybir as mybir

    F32 = mybir.dt.float32
    WDT = F32 if w_dtype_str == "f32" else mybir.dt.bfloat16

    nc = bacc.Bacc(None, target_bir_lowering=False)

    # ---- I/O ----
    xT = nc.dram_tensor("xT", [C + 128, T], F32, kind="ExternalInput")
    wihT = nc.dram_tensor("wihT", [C + 128, G4], F32, kind="ExternalInput")
    whhT = nc.dram_tensor("whhT", [C, G4], WDT, kind="ExternalInput")
    ident = nc.dram_tensor("ident", [4, 4], F32, kind="ExternalInput")
    y = nc.dram_tensor("y", [T, H], F32, kind="ExternalOutput")
    xg = nc.dram_tensor("xg", [T, G4], F32)  # internal staging, 32MB

    KTX = KT + 1  # 9 k-tiles incl bias row block

    from contextlib import ExitStack

    with ExitStack() as es_top:
        sems = {}
        for nm in ("ld_sem gp_sem ev_sem xw0_sem xw1_sem wh_sem ms_sem ps_sem ds_sem "
                   "as_sem cs_sem hs_sem tp_sem es_sem xr_sem yw0_sem yw1_sem "
                   "c2s_sem igs_sem").split():
            sems[nm] = es_top.enter_context(nc.semaphore(nm))
        ld_sem, gp_sem, ev_sem, wh_sem, ms_sem, ps_sem, ds_sem = (
            sems["ld_sem"], sems["gp_sem"], sems["ev_sem"],
            sems["wh_sem"], sems["ms_sem"], sems["ps_sem"], sems["ds_sem"])
        as_sem, cs_sem, hs_sem, tp_sem, es_sem, xr_sem = (
            sems["as_sem"], sems["cs_sem"], sems["hs_sem"], sems["tp_sem"],
            sems["es_sem"], sems["xr_sem"])
        xw_sems = (sems["xw0_sem"], sems["xw1_sem"])
        c2s_sem, igs_sem = sems["c2s_sem"], sems["igs_sem"]
        yw_sems = (sems["yw0_sem"], sems["yw1_sem"])
        # ---------------- Phase A: input GEMM ----------------
        with ExitStack() as es_a:
            xT_sb = es_a.enter_context(nc.sbuf_tensor("xT_sb", [128, KTX * T], F32))
            wih_sb = es_a.enter_context(nc.sbuf_tensor("wih_sb", [128, KTX * 2048], F32))
            stage = es_a.enter_context(nc.sbuf_tensor("stage", [128, 2 * 512], F32))
            gps = es_a.enter_context(nc.psum_tensor("gps", [128, 2 * 512], F32))
            blk = es_a.enter_context(nc.Block())
            # xT_sb k-tile k at cols [k*T, (k+1)*T); wih_sb k-tile at [k*2048, ..)
            @blk.sync
            def _(s):
                for k in range(KTX):
                    s.dma_start(
                        xT_sb[:, k * T : (k + 1) * T], xT[k * 128 : (k + 1) * 128, :]
                    ).then_inc(ld_sem, 16)
                # pass 0 wih halves
                for k in range(KTX):
                    s.dma_start(
                        wih_sb[:, k * 2048 : (k + 1) * 2048],
                        wihT[k * 128 : (k + 1) * 128, 0:2048],
                    ).then_inc(ld_sem, 16)
                # pass 1 loads gated on pass-0 compute done
                s.wait_ge(gp_sem, 64)
                for k in range(KTX):
                    s.dma_start(
                        wih_sb[:, k * 2048 : (k + 1) * 2048],
                        wihT[k * 128 : (k + 1) * 128, 2048:4096],
                    ).then_inc(ld_sem, 16)

            @blk.tensor
            def _(t):
                for p in range(2):
                    t.wait_ge(ld_sem, 16 * KTX * (2 + p))
                    for m in range(16):  # bt tiles of 128
                        for ns in range(4):  # 512-col chunks within the half
                            idx = p * 64 + m * 4 + ns
                            if idx >= 2:
                                t.wait_ge(ev_sem, idx - 1)
                            bank = idx % 2
                            for k in range(KTX):
                                t.matmul(
                                    gps[:, bank * 512 : (bank + 1) * 512],
                                    xT_sb[:, k * T + m * 128 : k * T + (m + 1) * 128],
                                    wih_sb[:, k * 2048 + ns * 512 : k * 2048 + (ns + 1) * 512],
                                    start=(k == 0),
                                    stop=(k == KTX - 1),
                                ).then_inc(gp_sem, 1) if k == KTX - 1 else t.matmul(
                                    gps[:, bank * 512 : (bank + 1) * 512],
                                    xT_sb[:, k * T + m * 128 : k * T + (m + 1) * 128],
                                    wih_sb[:, k * 2048 + ns * 512 : k * 2048 + (ns + 1) * 512],
                                    start=(k == 0),
                                    stop=(k == KTX - 1),
                                )

            @blk.vector
            def _(v):
                for p in range(2):
                    for m in range(16):
                        for ns in range(4):
                            idx = p * 64 + m * 4 + ns
                            bank = idx % 2
                            v.wait_ge(gp_sem, idx + 1)
                            if idx >= 2:
                                v.wait_ge(xw_sems[idx % 2], 16 * (idx // 2))
                            v.tensor_copy(
                                stage[:, bank * 512 : (bank + 1) * 512],
                                gps[:, bank * 512 : (bank + 1) * 512],
                            ).then_inc(ev_sem, 1)

            @blk.scalar
            def _(s):
                for p in range(2):
                    for m in range(16):
                        for ns in range(4):
                            idx = p * 64 + m * 4 + ns
                            bank = idx % 2
                            s.wait_ge(ev_sem, idx + 1)
                            s.dma_start(
                                xg[m * 128 : (m + 1) * 128, p * 2048 + ns * 512 : p * 2048 + (ns + 1) * 512],
                                stage[:, bank * 512 : (bank + 1) * 512],
                            ).then_inc(xw_sems[idx % 2], 16)
                s.wait_ge(xw_sems[0], 16 * 64)
                s.wait_ge(xw_sems[1], 16 * 64)

        # ---------------- Phase B: recurrence ----------------
        # wait: the matmul group semaphore convention above double-emits; see
        # the tensor block - it emits exactly one matmul per (k), with
        # then_inc only on the last. (Python ternary keeps one instruction.)
        LS = N_STEPS
        with ExitStack() as es_b:
            whh_sb = es_b.enter_context(nc.sbuf_tensor("whh_sb", [128, KT * G4], WDT))
            hT_pack = es_b.enter_context(nc.sbuf_tensor("hT_pack", [128, KT * 4], WDT))
            id_sb = es_b.enter_context(nc.sbuf_tensor("id_sb", [4, 4], F32))
            g_sb = es_b.enter_context(nc.sbuf_tensor("g_sb", [4, G4], F32))
            xg_sb = es_b.enter_context(nc.sbuf_tensor("xg_sb", [4, G4], F32))
            c_sb = es_b.enter_context(nc.sbuf_tensor("c_sb", [4, H], F32))
            c2_sb = es_b.enter_context(nc.sbuf_tensor("c2_sb", [4, H], F32))
            t_sb = es_b.enter_context(nc.sbuf_tensor("t_sb", [4, H], F32))
            h_sb = es_b.enter_context(nc.sbuf_tensor("h_sb", [4, 2 * H], F32))
            G_ps = es_b.enter_context(nc.psum_tensor("G_ps", [4, 6 * 512], F32))
            TA_ps = es_b.enter_context(nc.psum_tensor("TA_ps", [128, 16], F32))
            TB_ps = es_b.enter_context(nc.psum_tensor("TB_ps", [128, 16], F32))
            blk = es_b.enter_context(nc.Block())
            SIG = mybir.ActivationFunctionType.Sigmoid
            TANH = mybir.ActivationFunctionType.Tanh

            def gbank(n):  # psum column base for gate chunk n (chunks 6,7 alias 0,1)
                return (n % 6) * 512

            @blk.sync
            def _(s):
                for k in range(KT):
                    s.dma_start(
                        whh_sb[:, k * G4 : (k + 1) * G4], whhT[k * 128 : (k + 1) * 128, :]
                    ).then_inc(wh_sem, 16)
                s.dma_start(id_sb[:, :], ident[:, :]).then_inc(wh_sem, 16)
                for l in range(LS if not NO_DMA else 1):
                    if l >= 1:
                        # single xg buffer: refill after step l-1's adds consumed it
                        s.wait_ge(ds_sem, 8 * l)
                    s.dma_start(xg_sb[:, :], xg[4 * l : 4 * l + 4, :]).then_inc(
                        xr_sem, 16
                    )

            @blk.scalar
            def _(s):
                # y writeback on the scalar (HWDGE) queue + the ACT work
                for l in range(LS):
                    # ACT: A1 sigmoid(i,f), A2 tanh(g), A3 sigmoid(o), A4 tanh(c)
                    s.wait_ge(ds_sem, 8 * l + 4)
                    s.activation(g_sb[:, 0:2048], g_sb[:, 0:2048], SIG).then_inc(as_sem, 1)
                    s.wait_ge(ds_sem, 8 * l + 6)
                    s.activation(g_sb[:, 2048:3072], g_sb[:, 2048:3072], TANH).then_inc(as_sem, 1)
                    s.wait_ge(ds_sem, 8 * l + 8)
                    s.activation(g_sb[:, 3072:4096], g_sb[:, 3072:4096], SIG).then_inc(as_sem, 1)
                    s.wait_ge(cs_sem, l + 1)
                    s.activation(t_sb[:, :], c_sb[:, :], TANH).then_inc(as_sem, 1)
                    # y writeback of h(l)
                    if NO_DMA and l != LS - 1:
                        continue
                    s.wait_ge(hs_sem, l + 1)
                    s.dma_start(
                        y[4 * l : 4 * l + 4, :], h_sb[:, (l % 2) * H : (l % 2 + 1) * H]
                    ).then_inc(yw_sems[l % 2], 16)
                if not NO_DMA:
                    s.wait_ge(yw_sems[0], 16 * ((LS + 1) // 2))
                    s.wait_ge(yw_sems[1], 16 * (LS // 2))

            @blk.gpsimd
            def _(g):
                g.memset(hT_pack[:, :], 0.0)
                g.memset(c_sb[:, :], 0.0).then_inc(ms_sem, 1)

            @blk.tensor
            def _(t):
                t.wait_ge(wh_sem, 16 * (KT + 1))
                t.wait_ge(ms_sem, 1)
                for l in range(LS):
                    if l >= 1:
                        t.wait_ge(es_sem, 2 * l)  # hT(l-1) fully evacuated
                    if l >= 1:
                        t.wait_ge(ds_sem, 8 * (l - 1) + 6)  # banks 0..5 free
                    for n in range(8):
                        if n >= 6:
                            t.wait_ge(ds_sem, 8 * l + (n - 6) + 1)
                        for k in range(KT):
                            mm = t.matmul(
                                G_ps[:, gbank(n) : gbank(n) + 512],
                                hT_pack[:, k * 4 : k * 4 + 4],
                                whh_sb[:, k * G4 + n * 512 : k * G4 + (n + 1) * 512],
                                start=(k == 0),
                                stop=(k == KT - 1),
                            )
                            if k == KT - 1:
                                mm.then_inc(ps_sem, 1)
                    # transposes of h(l), two groups of 4 into banks A/B
                    t.wait_ge(hs_sem, l + 1)
                    for kk in range(KT):
                        tp = TA_ps if kk < 4 else TB_ps
                        ins = t.transpose(
                            tp[:, (kk % 4) * 4 : (kk % 4) * 4 + 4],
                            h_sb[:, (l % 2) * H + kk * 128 : (l % 2) * H + (kk + 1) * 128],
                            id_sb[:, :],
                        )
                        if kk % 4 == 3:
                            ins.then_inc(tp_sem, 1)

            @blk.vector
            def _(v):
                for l in range(LS):
                    # gate adds: g = G_ps + xg
                    for n in range(8):
                        v.wait_ge(ps_sem, 8 * l + n + 1)
                        if n == 0:
                            v.wait_ge(xr_sem, 16 * ((l + 1) if not NO_DMA else 1))
                        v.tensor_add(
                            g_sb[:, n * 512 : (n + 1) * 512],
                            G_ps[:, gbank(n) : gbank(n) + 512],
                            xg_sb[:, n * 512 : (n + 1) * 512],
                        ).then_inc(ds_sem, 1)
                    # c2 = f * c
                    v.wait_ge(as_sem, 4 * l + 1)
                    if l >= 1:
                        v.wait_ge(cs_sem, l)  # c_sb from step l-1 retired
                    v.tensor_mul(c2_sb[:, :], g_sb[:, 1024:2048], c_sb[:, :]).then_inc(
                        c2s_sem, 1
                    )
                    # t_sb reused as ig scratch: ig = i * g
                    v.wait_ge(as_sem, 4 * l + 2)
                    v.tensor_mul(t_sb[:, :], g_sb[:, 0:1024], g_sb[:, 2048:3072]).then_inc(
                        igs_sem, 1
                    )
                    # c = c2 + ig (same-engine RAW needs explicit sems)
                    v.wait_ge(c2s_sem, l + 1)
                    v.wait_ge(igs_sem, l + 1)
                    v.tensor_add(c_sb[:, :], c2_sb[:, :], t_sb[:, :]).then_inc(cs_sem, 1)
                    # h = o * tanh(c)
                    v.wait_ge(as_sem, 4 * l + 4)
                    if l >= 2 and not NO_DMA:
                        v.wait_ge(yw_sems[l % 2], 16 * (l // 2))
                    v.tensor_mul(
                        h_sb[:, (l % 2) * H : (l % 2 + 1) * H], g_sb[:, 3072:4096], t_sb[:, :]
                    ).then_inc(hs_sem, 1)
                    # hT evacs, 2 groups of 4 k-tiles
                    v.wait_ge(tp_sem, 2 * l + 1)
                    v.tensor_copy(hT_pack[:, 0:16], TA_ps[:, 0:16]).then_inc(es_sem, 1)
                    v.wait_ge(tp_sem, 2 * l + 2)
                    v.tensor_copy(hT_pack[:, 16:32], TB_ps[:, 0:16]).then_inc(es_sem, 1)

    nc.finalize()
    return nc


def _get_nc(w_dtype_str):
    if w_dtype_str not in _cached:
        _cached[w_dtype_str] = _build(w_dtype_str)
    return _cached[w_dtype_str]


W_DTYPE = os.environ.get("DLSTM_WDT", "f32")


def kernel(x, W_ih, W_hh, b_ih, b_hh):
    from concourse.bass_utils import run_bass_kernel_spmd
    import ml_dtypes

    x = np.asarray(x, np.float32)
    W_ih = np.asarray(W_ih, np.float32)
    W_hh = np.asarray(W_hh, np.float32)
    bias = (np.asarray(b_ih, np.float32) + np.asarray(b_hh, np.float32))

    nc = _get_nc(W_DTYPE)

    # host-side prep
    wihT_ext = np.zeros((C + 128, G4), np.float32)
    wihT_ext[:C] = W_ih.T
    wihT_ext[C] = bias
    whhT = W_hh.T.copy()
    if W_DTYPE == "bf16":
        whhT = whhT.astype(ml_dtypes.bfloat16)
    ident = np.eye(4, dtype=np.float32)

    in_maps = []
    for c in range(NCORES):
        xT_ext = np.zeros((C + 128, T), np.float32)
        xT_ext[:C] = x[c].T
        xT_ext[C] = 1.0
        in_maps.append(
            {"xT": xT_ext, "wihT": wihT_ext, "whhT": whhT, "ident": ident}
        )

    res = run_bass_kernel_spmd(nc, in_maps, list(range(NCORES)))
    out = np.stack([res.results[c]["y"] for c in range(NCORES)], axis=0)
    return out.astype(np.float32)
